# revision 7
# baseline (speedup 1.0000x reference)
"""BLSTM kernel for Trainium2 (8 NeuronCores, data-parallel over batch).

Problem: bidirectional LSTM, B=1024, T=512, V=128, H=128, HH=64.
  embedded = emb[x];  h_f = lstm_fwd(embedded);  h_b = lstm_bwd(embedded)
  out = concat(h_f, h_b) @ W_fc.T + b_fc

Design (per core, B_local = 128):
  * Everything "hidden-major": state tiles are [128, B] where the partition
    dim stacks [fwd 64 units ; bwd 64 units]. No transposes anywhere.
  * Input projections never touch the big matmul: since V=128, we precompute
    T4[u, v, g] = (emb @ W_ih_dir[gate g].T)[v, u] on device (4 small
    matmuls), and per step GPSIMD ap_gather fetches the per-token rows.
    The gather's per-16-partition index groups let the fwd half (partitions
    0:63) gather token x[:, t] while the bwd half (64:127) gathers
    x[:, T-1-t] in the same instruction.
  * One PSUM bank [128, 4, B] holds all 4 gate pre-activations: one
    identity matmul adds the gathered input gates, then 4 block-diagonal
    recurrent matmuls (lhsT const, rhs = h) accumulate W_hh @ h.
  * Single sigmoid over all gates: tanh is computed as 2*sigmoid(2x)-1 by
    scaling the g-gate weights by 2 and keeping the cell state doubled
    (chat = 2c):   chat' = s_f*chat + s_i*(4*s_g - 2)
                   h'    = s_o*(2*sigmoid(chat') - 1)
    The (a*x+b)*y forms are single fused DVE ops (affine_mul_reduce).

kernel(**inputs) takes the full unsharded inputs and returns the full
[1024, 128] float32 output; sharding/gather happens on the host.
"""

import os
import sys

sys.path.insert(0, "/opt/trn_rl_repo")

import numpy as np

HH, H, V, T, B, NCORES = 64, 128, 128, 512, 1024, 8
BL = B // NCORES  # 128 batch per core
GATE_REF = [0, 1, 3, 2]  # gate slot [i, f, o, g] -> reference row-block index
NSTEP_IDX = 8  # idx columns per step (BL/16)

_CACHE = {}


# --------------------------------------------------------------------------
# host-side packing (pure data movement / tiny reshapes, no model FLOPs)
# --------------------------------------------------------------------------

def _pack_consts(emb, W_ih_f, W_hh_f, W_ih_b, W_hh_b, W_fc, b_fc):
    f32 = np.float32
    consts = {}
    for s, r in enumerate(GATE_REF):
        scale = f32(2.0 if s == 3 else 1.0)
        wg = np.zeros((128, 128), f32)
        wg[:64, :64] = W_hh_f[r * 64:(r + 1) * 64]
        wg[64:, 64:] = W_hh_b[r * 64:(r + 1) * 64]
        consts[f"whhT{s}"] = np.ascontiguousarray((scale * wg).T)
        wi = np.concatenate(
            [W_ih_f[r * 64:(r + 1) * 64], W_ih_b[r * 64:(r + 1) * 64]], axis=0
        ).astype(f32)  # [128, H]
        consts[f"wihT{s}"] = np.ascontiguousarray((scale * wi).T)  # [H, 128]
    consts["embT"] = np.ascontiguousarray(emb.T.astype(f32))      # [H, V]
    consts["wfcT"] = np.ascontiguousarray(W_fc.T.astype(f32))     # [H, V]
    consts["bfc"] = np.ascontiguousarray(b_fc.reshape(V, 1).astype(f32))
    consts["ident"] = np.eye(128, dtype=f32)
    return consts


def _pack_idx(x_local):
    """x_local [BL, T] int32 -> wrapped gather indices [128, T*8] int16.

    Position j of a gather = batch j; idx lives at [16g + j%16, j//16].
    Groups 0-3 (fwd half) use x[:, t]; groups 4-7 (bwd) use x[:, T-1-t].
    """
    xl = np.asarray(x_local, dtype=np.int16)        # [BL, T]
    # w[p, t, s] = xl[s*16 + p, t]
    w = xl.reshape(NSTEP_IDX, 16, T).transpose(1, 2, 0)   # [16, T, 8]
    wf = w.reshape(16, T * NSTEP_IDX)
    wb = w[:, ::-1, :].reshape(16, T * NSTEP_IDX)
    idx = np.concatenate([np.tile(wf, (4, 1)), np.tile(wb, (4, 1))], axis=0)
    return np.ascontiguousarray(idx)                # [128, T*8]


# --------------------------------------------------------------------------
# device module
# --------------------------------------------------------------------------

def _build_module():
    import concourse.bacc as bacc
    import concourse.mybir as mybir
    import concourse.tile as tile

    f32 = mybir.dt.float32
    AF = mybir.ActivationFunctionType

    nc = bacc.Bacc(trn_type="TRN2", target_bir_lowering=False)

    d_whhT = [nc.dram_tensor(f"whhT{s}", [128, 128], f32, kind="ExternalInput")
              for s in range(4)]
    d_wihT = [nc.dram_tensor(f"wihT{s}", [H, 128], f32, kind="ExternalInput")
              for s in range(4)]
    d_embT = nc.dram_tensor("embT", [H, V], f32, kind="ExternalInput")
    d_wfcT = nc.dram_tensor("wfcT", [H, V], f32, kind="ExternalInput")
    d_bfc = nc.dram_tensor("bfc", [V, 1], f32, kind="ExternalInput")
    d_idx = nc.dram_tensor("idx", [128, T * NSTEP_IDX], mybir.dt.int16,
                           kind="ExternalInput")
    d_ident = nc.dram_tensor("ident", [128, 128], f32, kind="ExternalInput")
    d_out = nc.dram_tensor("outT", [V, BL], f32, kind="ExternalOutput")

    with tile.TileContext(nc) as tc:
        with (
            tc.tile_pool(name="const", bufs=1) as cpool,
            tc.tile_pool(name="state", bufs=2) as spool,
            tc.tile_pool(name="gin", bufs=4) as gpool,
            tc.tile_pool(name="sig", bufs=2) as sigpool,
            tc.tile_pool(name="work", bufs=3) as wpool,
            tc.tile_pool(name="psum", bufs=2, space="PSUM") as ppool,
            tc.tile_pool(name="psum1", bufs=1, space="PSUM") as ppool1,
        ):
            # ---- load constants ------------------------------------------
            whhT = []
            wihT = []
            for s in range(4):
                t_w = cpool.tile([128, 128], f32, tag=f"whhT{s}")
                nc.sync.dma_start(t_w[:], d_whhT[s][:])
                whhT.append(t_w)
                t_i = cpool.tile([H, 128], f32, tag=f"wihT{s}")
                nc.sync.dma_start(t_i[:], d_wihT[s][:])
                wihT.append(t_i)
            embT = cpool.tile([H, V], f32, tag="embT")
            nc.sync.dma_start(embT[:], d_embT[:])
            wfcT = cpool.tile([H, V], f32, tag="wfcT")
            nc.sync.dma_start(wfcT[:], d_wfcT[:])
            bfc = cpool.tile([V, 1], f32, tag="bfc")
            nc.sync.dma_start(bfc[:], d_bfc[:])
            idx = cpool.tile([128, T * NSTEP_IDX], mybir.dt.int16, tag="idx")
            nc.sync.dma_start(idx[:], d_idx[:])
            ident = cpool.tile([128, 128], f32, tag="ident")
            nc.sync.dma_start(ident[:], d_ident[:])

            # ---- input-projection table T4[u, v, g] ----------------------
            t4psum = ppool1.tile([128, 4, V], f32, tag="t4psum")
            for s in range(4):
                nc.tensor.matmul(t4psum[:, s, :], wihT[s][:], embT[:],
                                 start=True, stop=True)
            t4 = cpool.tile([128, V, 4], f32, tag="t4")
            nc.vector.tensor_copy(t4[:], t4psum[:].rearrange("p g v -> p v g"))

            # ---- state ---------------------------------------------------
            h = spool.tile([128, BL], f32, tag="h")
            chat = spool.tile([128, BL], f32, tag="chat")
            nc.vector.memset(h[:], 0.0)
            nc.vector.memset(chat[:], 0.0)

            # ---- recurrence ----------------------------------------------
            for t in range(T):
                gin = gpool.tile([128, BL, 4], f32, tag="gin")
                nc.gpsimd.ap_gather(
                    gin[:], t4[:], idx[:, t * NSTEP_IDX:(t + 1) * NSTEP_IDX],
                    channels=128, num_elems=V, d=4, num_idxs=BL,
                )
                g_ps = ppool.tile([128, 4, BL], f32, tag="g_ps")
                nc.tensor.matmul(g_ps[:], ident[:],
                                 gin[:].rearrange("p j g -> p g j"),
                                 start=True, stop=False)
                for s in range(4):
                    nc.tensor.matmul(g_ps[:, s, :], whhT[s][:], h[:],
                                     start=False, stop=(s == 3))
                sig = sigpool.tile([128, 4, BL], f32, tag="sig")
                nc.scalar.activation(sig[:], g_ps[:], AF.Sigmoid)

                p = wpool.tile([128, BL], f32, tag="p")
                acc0 = wpool.tile([128, 1], f32, tag="acc0")
                # p = (4*s_g - 2) * s_i
                nc.vector.affine_mul_reduce(p[:], acc0[:], sig[:, 3, :],
                                            sig[:, 0, :], 4.0, -2.0)
                q = wpool.tile([128, BL], f32, tag="q")
                nc.vector.tensor_mul(q[:], sig[:, 1, :], chat[:])
                chat_new = spool.tile([128, BL], f32, tag="chat")
                nc.vector.tensor_add(chat_new[:], p[:], q[:])
                chat = chat_new

                sc = wpool.tile([128, BL], f32, tag="sc")
                nc.scalar.activation(sc[:], chat[:], AF.Sigmoid)
                h_new = spool.tile([128, BL], f32, tag="h")
                acc1 = wpool.tile([128, 1], f32, tag="acc1")
                # h' = (2*s_c - 1) * s_o
                nc.vector.affine_mul_reduce(h_new[:], acc1[:], sc[:],
                                            sig[:, 2, :], 2.0, -1.0)
                h = h_new

            # ---- final projection ----------------------------------------
            out_ps = ppool1.tile([V, BL], f32, tag="out_ps")
            nc.tensor.matmul(out_ps[:], wfcT[:], h[:], start=True, stop=True)
            out_sb = wpool.tile([V, BL], f32, tag="out_sb")
            nc.scalar.activation(out_sb[:], out_ps[:], AF.Identity,
                                 bias=bfc[:, 0:1])
            nc.sync.dma_start(d_out[:], out_sb[:])

    nc.compile()
    return nc


def _get_module():
    if "nc" not in _CACHE:
        _CACHE["nc"] = _build_module()
    return _CACHE["nc"]


# --------------------------------------------------------------------------
# entry point
# --------------------------------------------------------------------------

def _get_runner():
    """Build (once) a jitted shard_map runner over the 8 cores, mirroring
    bass2jax.run_bass_via_pjrt but reusable across calls for timing."""
    if "runner" in _CACHE:
        return _CACHE["runner"]
    import jax
    import concourse.mybir as mybir
    from concourse import bass2jax
    from jax.sharding import Mesh, PartitionSpec
    from jax.experimental.shard_map import shard_map

    nc = _get_module()
    bass2jax.install_neuronx_cc_hook()
    partition_name = nc.partition_id_tensor.name if nc.partition_id_tensor else None
    in_names, out_names, out_avals, zero_shapes = [], [], [], []
    for alloc in nc.m.functions[0].allocations:
        if not isinstance(alloc, mybir.MemoryLocationSet):
            continue
        name = alloc.memorylocations[0].name
        if alloc.kind == "ExternalInput":
            if name != partition_name:
                in_names.append(name)
        elif alloc.kind == "ExternalOutput":
            shape = tuple(alloc.tensor_shape)
            dtype = mybir.dt.np(alloc.dtype)
            out_names.append(name)
            out_avals.append(jax.core.ShapedArray(shape, dtype))
            zero_shapes.append((shape, dtype))
    n_params = len(in_names)
    n_outs = len(out_names)
    all_in_names = list(in_names) + list(out_names)
    if partition_name is not None:
        all_in_names.append(partition_name)
    donate = tuple(range(n_params, n_params + n_outs))

    def _body(*args):
        operands = list(args)
        if partition_name is not None:
            operands.append(bass2jax.partition_id_tensor())
        outs = bass2jax._bass_exec_p.bind(
            *operands,
            out_avals=tuple(out_avals),
            in_names=tuple(all_in_names),
            out_names=tuple(out_names),
            lowering_input_output_aliases=(),
            sim_require_finite=True,
            sim_require_nnan=True,
            nc=nc,
        )
        return tuple(outs)

    devices = jax.devices()[:NCORES]
    mesh = Mesh(np.asarray(devices), ("core",))
    sharded = jax.jit(
        shard_map(_body, mesh=mesh,
                  in_specs=(PartitionSpec("core"),) * (n_params + n_outs),
                  out_specs=(PartitionSpec("core"),) * n_outs,
                  check_rep=False),
        donate_argnums=donate, keep_unused=True,
    )

    def run(in_maps):
        concat_in = [
            np.concatenate([np.asarray(in_maps[c][name]) for c in range(NCORES)],
                           axis=0)
            for name in in_names
        ]
        zeros = [np.zeros((NCORES * s[0], *s[1:]), d) for s, d in zero_shapes]
        out_arrs = sharded(*concat_in, *zeros)
        out_arrs = [np.asarray(a) for a in out_arrs]
        return [
            {name: out_arrs[i].reshape(NCORES, *zero_shapes[i][0])[c]
             for i, name in enumerate(out_names)}
            for c in range(NCORES)
        ]

    _CACHE["runner"] = run
    return run


def _make_in_maps(x, emb, W_ih_f, W_hh_f, W_ih_b, W_hh_b, W_fc, b_fc):
    consts = _pack_consts(
        np.asarray(emb, np.float32), np.asarray(W_ih_f, np.float32),
        np.asarray(W_hh_f, np.float32), np.asarray(W_ih_b, np.float32),
        np.asarray(W_hh_b, np.float32), np.asarray(W_fc, np.float32),
        np.asarray(b_fc, np.float32),
    )
    x = np.asarray(x)
    in_maps = []
    for c in range(NCORES):
        m = dict(consts)
        m["idx"] = _pack_idx(x[c * BL:(c + 1) * BL, :])
        in_maps.append(m)
    return in_maps


def kernel(x, lengths, emb, W_ih_f, W_hh_f, W_ih_b, W_hh_b, W_fc, b_fc):
    in_maps = _make_in_maps(x, emb, W_ih_f, W_hh_f, W_ih_b, W_hh_b, W_fc, b_fc)
    results = _get_runner()(in_maps)
    out = np.concatenate(
        [np.ascontiguousarray(results[c]["outT"].T) for c in range(NCORES)],
        axis=0,
    ).astype(np.float32)
    return out


# revision 8
# speedup vs baseline: 3.2988x; 3.2988x over previous
"""BLSTM kernel for Trainium2 (8 NeuronCores, data-parallel over batch).

Problem: bidirectional LSTM, B=1024, T=512, V=128, H=128, HH=64.
  embedded = emb[x];  h_f = lstm_fwd(embedded);  h_b = lstm_bwd(embedded)
  out = concat(h_f, h_b) @ W_fc.T + b_fc

Design (per core, B_local = 128):
  * Everything "hidden-major": state tiles are [128, B] where the partition
    dim stacks [fwd 64 units ; bwd 64 units]. No transposes anywhere.
  * Input projections never touch the big matmul: since V=128, we precompute
    T4[u, v, g] = (emb @ W_ih_dir[gate g].T)[v, u] on device (4 small
    matmuls), and per step GPSIMD ap_gather fetches the per-token rows.
    The gather's per-16-partition index groups let the fwd half (partitions
    0:63) gather token x[:, t] while the bwd half (64:127) gathers
    x[:, T-1-t] in the same instruction.
  * One PSUM bank [128, 4, B] holds all 4 gate pre-activations: one
    identity matmul adds the gathered input gates, then 4 block-diagonal
    recurrent matmuls (lhsT const, rhs = h) accumulate W_hh @ h.
  * Single sigmoid over all gates: tanh is computed as 2*sigmoid(2x)-1 by
    scaling the g-gate weights by 2 and keeping the cell state doubled
    (chat = 2c):   chat' = s_f*chat + s_i*(4*s_g - 2)
                   h'    = s_o*(2*sigmoid(chat') - 1)
    The (a*x+b)*y forms are single fused DVE ops (affine_mul_reduce).

kernel(**inputs) takes the full unsharded inputs and returns the full
[1024, 128] float32 output; sharding/gather happens on the host.
"""

import os
import sys

sys.path.insert(0, "/opt/trn_rl_repo")

import numpy as np

HH, H, V, T, B, NCORES = 64, 128, 128, 512, 1024, 8
BL = B // NCORES  # 128 batch per core
GATE_REF = [0, 1, 3, 2]  # gate slot [i, f, o, g] -> reference row-block index
NSTEP_IDX = 8  # idx columns per step (BL/16)

_CACHE = {}


# --------------------------------------------------------------------------
# host-side packing (pure data movement / tiny reshapes, no model FLOPs)
# --------------------------------------------------------------------------

def _pack_consts(emb, W_ih_f, W_hh_f, W_ih_b, W_hh_b, W_fc, b_fc):
    f32 = np.float32
    consts = {}
    for s, r in enumerate(GATE_REF):
        scale = f32(2.0 if s == 3 else 1.0)
        wg = np.zeros((128, 128), f32)
        wg[:64, :64] = W_hh_f[r * 64:(r + 1) * 64]
        wg[64:, 64:] = W_hh_b[r * 64:(r + 1) * 64]
        consts[f"whhT{s}"] = np.ascontiguousarray((scale * wg).T)
        wi = np.concatenate(
            [W_ih_f[r * 64:(r + 1) * 64], W_ih_b[r * 64:(r + 1) * 64]], axis=0
        ).astype(f32)  # [128, H]
        consts[f"wihT{s}"] = np.ascontiguousarray((scale * wi).T)  # [H, 128]
    consts["embT"] = np.ascontiguousarray(emb.T.astype(f32))      # [H, V]
    consts["wfcT"] = np.ascontiguousarray(W_fc.T.astype(f32))     # [H, V]
    consts["bfc"] = np.ascontiguousarray(b_fc.reshape(V, 1).astype(f32))
    consts["ident"] = np.eye(128, dtype=f32)
    return consts


def _pack_idx(x_local):
    """x_local [BL, T] int32 -> wrapped gather indices [128, T*8] int16.

    Position j of a gather = batch j; idx lives at [16g + j%16, j//16].
    Groups 0-3 (fwd half) use x[:, t]; groups 4-7 (bwd) use x[:, T-1-t].
    """
    xl = np.asarray(x_local, dtype=np.int16)        # [BL, T]
    # w[p, t, s] = xl[s*16 + p, t]
    w = xl.reshape(NSTEP_IDX, 16, T).transpose(1, 2, 0)   # [16, T, 8]
    wf = w.reshape(16, T * NSTEP_IDX)
    wb = w[:, ::-1, :].reshape(16, T * NSTEP_IDX)
    idx = np.concatenate([np.tile(wf, (4, 1)), np.tile(wb, (4, 1))], axis=0)
    return np.ascontiguousarray(idx)                # [128, T*8]


# --------------------------------------------------------------------------
# device module
# --------------------------------------------------------------------------

def _build_module():
    import concourse.bacc as bacc
    import concourse.mybir as mybir
    import concourse.tile as tile

    f32 = mybir.dt.float32
    AF = mybir.ActivationFunctionType

    nc = bacc.Bacc(trn_type="TRN2", target_bir_lowering=False)

    d_whhT = [nc.dram_tensor(f"whhT{s}", [128, 128], f32, kind="ExternalInput")
              for s in range(4)]
    d_wihT = [nc.dram_tensor(f"wihT{s}", [H, 128], f32, kind="ExternalInput")
              for s in range(4)]
    d_embT = nc.dram_tensor("embT", [H, V], f32, kind="ExternalInput")
    d_wfcT = nc.dram_tensor("wfcT", [H, V], f32, kind="ExternalInput")
    d_bfc = nc.dram_tensor("bfc", [V, 1], f32, kind="ExternalInput")
    d_idx = nc.dram_tensor("idx", [128, T * NSTEP_IDX], mybir.dt.int16,
                           kind="ExternalInput")
    d_ident = nc.dram_tensor("ident", [128, 128], f32, kind="ExternalInput")
    d_out = nc.dram_tensor("outT", [V, BL], f32, kind="ExternalOutput")

    with tile.TileContext(nc) as tc:
        with (
            tc.tile_pool(name="const", bufs=1) as cpool,
            tc.tile_pool(name="state", bufs=2) as spool,
            tc.tile_pool(name="gin", bufs=4) as gpool,
            tc.tile_pool(name="sig", bufs=2) as sigpool,
            tc.tile_pool(name="work", bufs=3) as wpool,
            tc.tile_pool(name="psum", bufs=2, space="PSUM") as ppool,
            tc.tile_pool(name="psum1", bufs=1, space="PSUM") as ppool1,
        ):
            # ---- load constants ------------------------------------------
            whhT = []
            wihT = []
            for s in range(4):
                t_w = cpool.tile([128, 128], f32, tag=f"whhT{s}")
                nc.sync.dma_start(t_w[:], d_whhT[s][:])
                whhT.append(t_w)
                t_i = cpool.tile([H, 128], f32, tag=f"wihT{s}")
                nc.sync.dma_start(t_i[:], d_wihT[s][:])
                wihT.append(t_i)
            embT = cpool.tile([H, V], f32, tag="embT")
            nc.sync.dma_start(embT[:], d_embT[:])
            wfcT = cpool.tile([H, V], f32, tag="wfcT")
            nc.sync.dma_start(wfcT[:], d_wfcT[:])
            bfc = cpool.tile([V, 1], f32, tag="bfc")
            nc.sync.dma_start(bfc[:], d_bfc[:])
            idx = cpool.tile([128, T * NSTEP_IDX], mybir.dt.int16, tag="idx")
            nc.sync.dma_start(idx[:], d_idx[:])
            ident = cpool.tile([128, 128], f32, tag="ident")
            nc.sync.dma_start(ident[:], d_ident[:])

            # ---- input-projection table T4[u, v, g] ----------------------
            t4psum = ppool1.tile([128, 4, V], f32, tag="t4psum")
            for s in range(4):
                nc.tensor.matmul(t4psum[:, s, :], wihT[s][:], embT[:],
                                 start=True, stop=True)
            t4 = cpool.tile([128, V, 4], f32, tag="t4")
            nc.vector.tensor_copy(t4[:], t4psum[:].rearrange("p g v -> p v g"))

            # ---- state ---------------------------------------------------
            h = spool.tile([128, BL], f32, tag="h")
            chat = spool.tile([128, BL], f32, tag="chat")
            nc.vector.memset(h[:], 0.0)
            nc.vector.memset(chat[:], 0.0)

            # ---- recurrence ----------------------------------------------
            for t in range(T):
                gin = gpool.tile([128, BL, 4], f32, tag="gin")
                nc.gpsimd.ap_gather(
                    gin[:], t4[:], idx[:, t * NSTEP_IDX:(t + 1) * NSTEP_IDX],
                    channels=128, num_elems=V, d=4, num_idxs=BL,
                )
                g_ps = ppool.tile([128, 4, BL], f32, tag="g_ps")
                nc.tensor.matmul(g_ps[:], ident[:],
                                 gin[:].rearrange("p j g -> p g j"),
                                 start=True, stop=False)
                for s in range(4):
                    nc.tensor.matmul(g_ps[:, s, :], whhT[s][:], h[:],
                                     start=False, stop=(s == 3))
                sig = sigpool.tile([128, 4, BL], f32, tag="sig")
                nc.scalar.activation(sig[:], g_ps[:], AF.Sigmoid)

                p = wpool.tile([128, BL], f32, tag="p")
                acc0 = wpool.tile([128, 1], f32, tag="acc0")
                # p = (4*s_g - 2) * s_i
                nc.vector.affine_mul_reduce(p[:], acc0[:], sig[:, 3, :],
                                            sig[:, 0, :], 4.0, -2.0)
                q = wpool.tile([128, BL], f32, tag="q")
                nc.vector.tensor_mul(q[:], sig[:, 1, :], chat[:])
                chat_new = spool.tile([128, BL], f32, tag="chat")
                nc.vector.tensor_add(chat_new[:], p[:], q[:])
                chat = chat_new

                sc = wpool.tile([128, BL], f32, tag="sc")
                nc.scalar.activation(sc[:], chat[:], AF.Sigmoid)
                h_new = spool.tile([128, BL], f32, tag="h")
                acc1 = wpool.tile([128, 1], f32, tag="acc1")
                # h' = (2*s_c - 1) * s_o
                nc.vector.affine_mul_reduce(h_new[:], acc1[:], sc[:],
                                            sig[:, 2, :], 2.0, -1.0)
                h = h_new

            # ---- final projection ----------------------------------------
            out_ps = ppool1.tile([V, BL], f32, tag="out_ps")
            nc.tensor.matmul(out_ps[:], wfcT[:], h[:], start=True, stop=True)
            out_sb = wpool.tile([V, BL], f32, tag="out_sb")
            nc.scalar.activation(out_sb[:], out_ps[:], AF.Identity,
                                 bias=bfc[:, 0:1])
            nc.sync.dma_start(d_out[:], out_sb[:])

    nc.compile()
    return nc


def _get_module():
    if "nc" not in _CACHE:
        _CACHE["nc"] = _build_module()
    return _CACHE["nc"]


# --------------------------------------------------------------------------
# entry point
# --------------------------------------------------------------------------

def _get_runner():
    """Build (once) a jitted shard_map runner over the 8 cores, mirroring
    bass2jax.run_bass_via_pjrt but reusable across calls for timing."""
    if "runner" in _CACHE:
        return _CACHE["runner"]
    import jax
    import concourse.mybir as mybir
    from concourse import bass2jax
    from jax.sharding import Mesh, PartitionSpec
    from jax.experimental.shard_map import shard_map

    nc = _get_module()
    bass2jax.install_neuronx_cc_hook()
    partition_name = nc.partition_id_tensor.name if nc.partition_id_tensor else None
    in_names, out_names, out_avals, zero_shapes = [], [], [], []
    for alloc in nc.m.functions[0].allocations:
        if not isinstance(alloc, mybir.MemoryLocationSet):
            continue
        name = alloc.memorylocations[0].name
        if alloc.kind == "ExternalInput":
            if name != partition_name:
                in_names.append(name)
        elif alloc.kind == "ExternalOutput":
            shape = tuple(alloc.tensor_shape)
            dtype = mybir.dt.np(alloc.dtype)
            out_names.append(name)
            out_avals.append(jax.core.ShapedArray(shape, dtype))
            zero_shapes.append((shape, dtype))
    n_params = len(in_names)
    n_outs = len(out_names)
    all_in_names = list(in_names) + list(out_names)
    if partition_name is not None:
        all_in_names.append(partition_name)
    donate = tuple(range(n_params, n_params + n_outs))

    def _body(*args):
        operands = list(args)
        if partition_name is not None:
            operands.append(bass2jax.partition_id_tensor())
        outs = bass2jax._bass_exec_p.bind(
            *operands,
            out_avals=tuple(out_avals),
            in_names=tuple(all_in_names),
            out_names=tuple(out_names),
            lowering_input_output_aliases=(),
            sim_require_finite=True,
            sim_require_nnan=True,
            nc=nc,
        )
        return tuple(outs)

    devices = jax.devices()[:NCORES]
    mesh = Mesh(np.asarray(devices), ("core",))
    sharded = jax.jit(
        shard_map(_body, mesh=mesh,
                  in_specs=(PartitionSpec("core"),) * (n_params + n_outs),
                  out_specs=(PartitionSpec("core"),) * n_outs,
                  check_rep=False),
        donate_argnums=donate, keep_unused=True,
    )

    in_sharding = jax.sharding.NamedSharding(mesh, PartitionSpec("core"))

    def run(in_maps, reuse_inputs=False):
        if reuse_inputs and "dev_in" in _CACHE:
            dev_in = _CACHE["dev_in"]
        else:
            concat_in = [
                np.concatenate(
                    [np.asarray(in_maps[c][name]) for c in range(NCORES)], axis=0)
                for name in in_names
            ]
            dev_in = [jax.device_put(a, in_sharding) for a in concat_in]
            _CACHE["dev_in"] = dev_in
        zeros = [
            jax.device_put(np.zeros((NCORES * s[0], *s[1:]), d), in_sharding)
            for s, d in zero_shapes
        ]
        out_arrs = sharded(*dev_in, *zeros)
        out_arrs = [np.asarray(a) for a in out_arrs]
        return [
            {name: out_arrs[i].reshape(NCORES, *zero_shapes[i][0])[c]
             for i, name in enumerate(out_names)}
            for c in range(NCORES)
        ]

    _CACHE["runner"] = run
    return run


def _make_in_maps(x, emb, W_ih_f, W_hh_f, W_ih_b, W_hh_b, W_fc, b_fc):
    consts = _pack_consts(
        np.asarray(emb, np.float32), np.asarray(W_ih_f, np.float32),
        np.asarray(W_hh_f, np.float32), np.asarray(W_ih_b, np.float32),
        np.asarray(W_hh_b, np.float32), np.asarray(W_fc, np.float32),
        np.asarray(b_fc, np.float32),
    )
    x = np.asarray(x)
    in_maps = []
    for c in range(NCORES):
        m = dict(consts)
        m["idx"] = _pack_idx(x[c * BL:(c + 1) * BL, :])
        in_maps.append(m)
    return in_maps


def kernel(x, lengths, emb, W_ih_f, W_hh_f, W_ih_b, W_hh_b, W_fc, b_fc):
    in_maps = _make_in_maps(x, emb, W_ih_f, W_hh_f, W_ih_b, W_hh_b, W_fc, b_fc)
    results = _get_runner()(in_maps)
    out = np.concatenate(
        [np.ascontiguousarray(results[c]["outT"].T) for c in range(NCORES)],
        axis=0,
    ).astype(np.float32)
    return out


# revision 22
# speedup vs baseline: 176.9681x; 53.6463x over previous
"""BLSTM kernel for Trainium2 (8 NeuronCores, data-parallel over batch).

Problem: bidirectional LSTM, B=1024, T=512, V=128, H=128, HH=64.
  embedded = emb[x];  h_f = lstm_fwd(embedded);  h_b = lstm_bwd(embedded)
  out = concat(h_f, h_b) @ W_fc.T + b_fc

Design (per core, B_local = 128):
  * Everything "hidden-major": state tiles are [128, B] where the partition
    dim stacks [fwd 64 units ; bwd 64 units]. No transposes anywhere.
  * Input projections never touch the big matmul: since V=128, we precompute
    T4[u, v, g] = (emb @ W_ih_dir[gate g].T)[v, u] on device (4 small
    matmuls), and per step GPSIMD ap_gather fetches the per-token rows.
    The gather's per-16-partition index groups let the fwd half (partitions
    0:63) gather token x[:, t] while the bwd half (64:127) gathers
    x[:, T-1-t] in the same instruction.
  * One PSUM bank [128, 4, B] holds all 4 gate pre-activations: one
    identity matmul adds the gathered input gates, then 4 block-diagonal
    recurrent matmuls (lhsT const, rhs = h) accumulate W_hh @ h.
  * Single sigmoid over all gates: tanh is computed as 2*sigmoid(2x)-1 by
    scaling the g-gate weights by 2 and keeping the cell state doubled
    (chat = 2c):   chat' = s_f*chat + s_i*(4*s_g - 2)
                   h'    = s_o*(2*sigmoid(chat') - 1)
    The (a*x+b)*y forms are single fused DVE ops (affine_mul_reduce).

kernel(**inputs) takes the full unsharded inputs and returns the full
[1024, 128] float32 output; sharding/gather happens on the host.
"""

import os
import sys

sys.path.insert(0, "/opt/trn_rl_repo")

import numpy as np

HH, H, V, T, B, NCORES = 64, 128, 128, 512, 1024, 8
BL = B // NCORES  # 128 batch per core
# gate slot order [i, f, g, o]; slot 2 (cell gate g) is pre-scaled by 2 so a
# single sigmoid gives tanh via 2*sigmoid(2x)-1
GATE_REF = [0, 1, 2, 3]
GATE_SCALED = 2  # slot whose weights are doubled
NSTEP_IDX = 8  # idx columns per step (BL/16)

_CACHE = {}


# --------------------------------------------------------------------------
# host-side packing (pure data movement / tiny reshapes, no model FLOPs)
# --------------------------------------------------------------------------

def _pack_consts(emb, W_ih_f, W_hh_f, W_ih_b, W_hh_b, W_fc, b_fc):
    f32 = np.float32
    try:
        from ml_dtypes import bfloat16
    except ImportError:  # pragma: no cover
        import jax.numpy as jnp
        bfloat16 = jnp.bfloat16
    consts = {}
    for s, r in enumerate(GATE_REF):
        scale = f32(2.0 if s == GATE_SCALED else 1.0)
        wg = np.zeros((128, 128), f32)
        wg[:64, :64] = W_hh_f[r * 64:(r + 1) * 64]
        wg[64:, 64:] = W_hh_b[r * 64:(r + 1) * 64]
        consts[f"whhT{s}"] = ((scale * wg).T).astype(bfloat16)
        wi = np.concatenate(
            [W_ih_f[r * 64:(r + 1) * 64], W_ih_b[r * 64:(r + 1) * 64]], axis=0
        ).astype(f32)  # [128, H]
        consts[f"wihT{s}"] = np.ascontiguousarray((scale * wi).T)  # [H, 128]
    consts["embT"] = np.ascontiguousarray(emb.T.astype(f32))      # [H, V]
    consts["wfcT"] = np.ascontiguousarray(W_fc.T.astype(f32))     # [H, V]
    consts["bfc"] = np.ascontiguousarray(b_fc.reshape(V, 1).astype(f32))
    consts["ident"] = np.eye(128, dtype=bfloat16)
    return consts


def _pack_idx(x_local):
    """x_local [BL, T] int32 -> wrapped gather indices [128, T*8] int16.

    Position j of a gather = batch j; idx lives at [16g + j%16, j//16].
    Groups 0-3 (fwd half) use x[:, t]; groups 4-7 (bwd) use x[:, T-1-t].
    """
    xl = np.asarray(x_local, dtype=np.int16)        # [BL, T]
    # w[p, t, s] = xl[s*16 + p, t]
    w = xl.reshape(NSTEP_IDX, 16, T).transpose(1, 2, 0)   # [16, T, 8]
    wf = w.reshape(16, T * NSTEP_IDX)
    wb = w[:, ::-1, :].reshape(16, T * NSTEP_IDX)
    idx = np.concatenate([np.tile(wf, (4, 1)), np.tile(wb, (4, 1))], axis=0)
    return np.ascontiguousarray(idx)                # [128, T*8]


# --------------------------------------------------------------------------
# device module
# --------------------------------------------------------------------------

def _build_module(reps=1):
    import concourse.bacc as bacc
    import concourse.mybir as mybir
    import concourse.tile as tile

    f32 = mybir.dt.float32
    bf16 = mybir.dt.bfloat16
    AF = mybir.ActivationFunctionType

    nc = bacc.Bacc(trn_type="TRN2", target_bir_lowering=False)

    d_whhT = [nc.dram_tensor(f"whhT{s}", [128, 128], bf16, kind="ExternalInput")
              for s in range(4)]
    d_wihT = [nc.dram_tensor(f"wihT{s}", [H, 128], f32, kind="ExternalInput")
              for s in range(4)]
    d_embT = nc.dram_tensor("embT", [H, V], f32, kind="ExternalInput")
    d_wfcT = nc.dram_tensor("wfcT", [H, V], f32, kind="ExternalInput")
    d_bfc = nc.dram_tensor("bfc", [V, 1], f32, kind="ExternalInput")
    d_idx = nc.dram_tensor("idx", [128, T * NSTEP_IDX], mybir.dt.int16,
                           kind="ExternalInput")
    d_ident = nc.dram_tensor("ident", [128, 128], bf16, kind="ExternalInput")
    d_out = nc.dram_tensor("outT", [V, BL], f32, kind="ExternalOutput")

    with tile.TileContext(nc) as tc:
        with (
            tc.tile_pool(name="const", bufs=1) as cpool,
            tc.tile_pool(name="state", bufs=2) as spool,
            tc.tile_pool(name="gin", bufs=4) as gpool,
            tc.tile_pool(name="sig", bufs=2) as sigpool,
            tc.tile_pool(name="work", bufs=3) as wpool,
            tc.tile_pool(name="psum", bufs=2, space="PSUM") as ppool,
            tc.tile_pool(name="psum1", bufs=1, space="PSUM") as ppool1,
        ):
            # ---- load constants ------------------------------------------
            whhT = []
            wihT = []
            for s in range(4):
                t_w = cpool.tile([128, 128], bf16, tag=f"whhT{s}")
                nc.sync.dma_start(t_w[:], d_whhT[s][:])
                whhT.append(t_w)
                t_i = cpool.tile([H, 128], f32, tag=f"wihT{s}")
                nc.sync.dma_start(t_i[:], d_wihT[s][:])
                wihT.append(t_i)
            embT = cpool.tile([H, V], f32, tag="embT")
            nc.sync.dma_start(embT[:], d_embT[:])
            wfcT32 = cpool.tile([H, V], f32, tag="wfcT")
            nc.sync.dma_start(wfcT32[:], d_wfcT[:])
            bfc = cpool.tile([V, 1], f32, tag="bfc")
            nc.sync.dma_start(bfc[:], d_bfc[:])
            idx = cpool.tile([128, T * NSTEP_IDX], mybir.dt.int16, tag="idx")
            nc.sync.dma_start(idx[:], d_idx[:])
            ident = cpool.tile([128, 128], bf16, tag="ident")
            nc.sync.dma_start(ident[:], d_ident[:])

            # ---- input-projection table T4[u, v, g] ----------------------
            t4psum = ppool1.tile([128, 4, V], f32, tag="t4psum")
            for s in range(4):
                nc.tensor.matmul(t4psum[:, s, :], wihT[s][:], embT[:],
                                 start=True, stop=True)
            t4 = cpool.tile([128, V, 4], bf16, tag="t4")
            nc.vector.tensor_copy(t4[:], t4psum[:].rearrange("p g v -> p v g"))

            # ---- state ---------------------------------------------------
            for _rep in range(reps):
              h = spool.tile([128, BL], bf16, tag="h")
              chat = spool.tile([128, BL], f32, tag="chat")
              nc.vector.memset(h[:], 0.0)
              nc.vector.memset(chat[:], 0.0)

              # ---- recurrence --------------------------------------------
              for t in range(T):
                gin = gpool.tile([128, BL, 4], bf16, tag="gin")
                nc.gpsimd.ap_gather(
                    gin[:], t4[:], idx[:, t * NSTEP_IDX:(t + 1) * NSTEP_IDX],
                    channels=128, num_elems=V, d=4, num_idxs=BL,
                )
                g_ps = ppool.tile([128, 4, BL], f32, tag="g_ps")
                nc.tensor.matmul(g_ps[:], ident[:],
                                 gin[:].rearrange("p j g -> p g j"),
                                 start=True, stop=False)
                for s in range(4):
                    nc.tensor.matmul(g_ps[:, s, :], whhT[s][:], h[:],
                                     start=False, stop=(s == 3))
                sig = sigpool.tile([128, 4, BL], f32, tag="sig")
                # sigmoid over [i, f, g] (the serial-chain gates) first,
                # o separately so the DVE chain can start sooner
                nc.scalar.activation(sig[:, 0:3, :], g_ps[:, 0:3, :],
                                     AF.Sigmoid)
                nc.scalar.activation(sig[:, 3, :], g_ps[:, 3, :], AF.Sigmoid)

                p = wpool.tile([128, BL], f32, tag="p")
                acc0 = wpool.tile([128, 1], f32, tag="acc0")
                # p = (4*s_g - 2) * s_i   (= 2*sigma_i*tanh(g))
                nc.vector.affine_mul_reduce(p[:], acc0[:], sig[:, 2, :],
                                            sig[:, 0, :], 4.0, -2.0)
                q = wpool.tile([128, BL], f32, tag="q")
                nc.vector.tensor_mul(q[:], sig[:, 1, :], chat[:])
                chat_new = spool.tile([128, BL], f32, tag="chat")
                nc.vector.tensor_add(chat_new[:], p[:], q[:])
                chat = chat_new

                sc = wpool.tile([128, BL], f32, tag="sc")
                nc.scalar.activation(sc[:], chat[:], AF.Sigmoid)
                h_new = spool.tile([128, BL], bf16, tag="h")
                acc1 = wpool.tile([128, 1], f32, tag="acc1")
                # h' = (2*s_c - 1) * s_o   (= sigma_o * tanh(c'))
                nc.vector.affine_mul_reduce(h_new[:], acc1[:], sc[:],
                                            sig[:, 3, :], 2.0, -1.0)
                h = h_new

            # ---- final projection (fp32 h for output precision) ----------
            h32 = wpool.tile([128, BL], f32, tag="h32")
            acc2 = wpool.tile([128, 1], f32, tag="acc2")
            nc.vector.affine_mul_reduce(h32[:], acc2[:], sc[:],
                                        sig[:, 3, :], 2.0, -1.0)
            out_ps = ppool1.tile([V, BL], f32, tag="out_ps")
            nc.tensor.matmul(out_ps[:], wfcT32[:], h32[:], start=True,
                             stop=True)
            out_sb = wpool.tile([V, BL], f32, tag="out_sb")
            nc.scalar.activation(out_sb[:], out_ps[:], AF.Identity,
                                 bias=bfc[:, 0:1])
            nc.sync.dma_start(d_out[:], out_sb[:])

    nc.compile()
    return nc


def _get_module(reps=1):
    key = f"nc{reps}"
    if key not in _CACHE:
        _CACHE[key] = _build_module(reps)
    return _CACHE[key]


# --------------------------------------------------------------------------
# entry point
# --------------------------------------------------------------------------

def _get_runner(reps=1):
    """Build (once) a jitted shard_map runner over the 8 cores, mirroring
    bass2jax.run_bass_via_pjrt but reusable across calls for timing."""
    rkey = f"runner{reps}"
    if rkey in _CACHE:
        return _CACHE[rkey]
    import jax
    import concourse.mybir as mybir
    from concourse import bass2jax
    from jax.sharding import Mesh, PartitionSpec
    from jax.experimental.shard_map import shard_map

    nc = _get_module(reps)
    bass2jax.install_neuronx_cc_hook()
    partition_name = nc.partition_id_tensor.name if nc.partition_id_tensor else None
    in_names, out_names, out_avals, zero_shapes = [], [], [], []
    for alloc in nc.m.functions[0].allocations:
        if not isinstance(alloc, mybir.MemoryLocationSet):
            continue
        name = alloc.memorylocations[0].name
        if alloc.kind == "ExternalInput":
            if name != partition_name:
                in_names.append(name)
        elif alloc.kind == "ExternalOutput":
            shape = tuple(alloc.tensor_shape)
            dtype = mybir.dt.np(alloc.dtype)
            out_names.append(name)
            out_avals.append(jax.core.ShapedArray(shape, dtype))
            zero_shapes.append((shape, dtype))
    n_params = len(in_names)
    n_outs = len(out_names)
    all_in_names = list(in_names) + list(out_names)
    if partition_name is not None:
        all_in_names.append(partition_name)
    donate = tuple(range(n_params, n_params + n_outs))

    def _body(*args):
        operands = list(args)
        if partition_name is not None:
            operands.append(bass2jax.partition_id_tensor())
        outs = bass2jax._bass_exec_p.bind(
            *operands,
            out_avals=tuple(out_avals),
            in_names=tuple(all_in_names),
            out_names=tuple(out_names),
            lowering_input_output_aliases=(),
            sim_require_finite=True,
            sim_require_nnan=True,
            nc=nc,
        )
        return tuple(outs)

    devices = jax.devices()[:NCORES]
    mesh = Mesh(np.asarray(devices), ("core",))
    sharded = jax.jit(
        shard_map(_body, mesh=mesh,
                  in_specs=(PartitionSpec("core"),) * (n_params + n_outs),
                  out_specs=(PartitionSpec("core"),) * n_outs,
                  check_rep=False),
        donate_argnums=donate, keep_unused=True,
    )

    in_sharding = jax.sharding.NamedSharding(mesh, PartitionSpec("core"))

    def run(in_maps, reuse_inputs=False):
        if reuse_inputs and "dev_in" in _CACHE:
            dev_in = _CACHE["dev_in"]
        else:
            concat_in = [
                np.concatenate(
                    [np.asarray(in_maps[c][name]) for c in range(NCORES)], axis=0)
                for name in in_names
            ]
            dev_in = [jax.device_put(a, in_sharding) for a in concat_in]
            _CACHE["dev_in"] = dev_in
        zeros = [
            jax.device_put(np.zeros((NCORES * s[0], *s[1:]), d), in_sharding)
            for s, d in zero_shapes
        ]
        out_arrs = sharded(*dev_in, *zeros)
        out_arrs = [np.asarray(a) for a in out_arrs]
        return [
            {name: out_arrs[i].reshape(NCORES, *zero_shapes[i][0])[c]
             for i, name in enumerate(out_names)}
            for c in range(NCORES)
        ]

    def timed(iters=6):
        import time as _time
        dev_in = _CACHE["dev_in"]
        times = []
        for _ in range(iters):
            zeros = [
                jax.device_put(np.zeros((NCORES * s[0], *s[1:]), d), in_sharding)
                for s, d in zero_shapes
            ]
            t0 = _time.perf_counter()
            r = sharded(*dev_in, *zeros)
            jax.block_until_ready(r)
            times.append(_time.perf_counter() - t0)
        return times

    run.timed = timed
    _CACHE[rkey] = run
    return run


def _make_in_maps(x, emb, W_ih_f, W_hh_f, W_ih_b, W_hh_b, W_fc, b_fc):
    consts = _pack_consts(
        np.asarray(emb, np.float32), np.asarray(W_ih_f, np.float32),
        np.asarray(W_hh_f, np.float32), np.asarray(W_ih_b, np.float32),
        np.asarray(W_hh_b, np.float32), np.asarray(W_fc, np.float32),
        np.asarray(b_fc, np.float32),
    )
    x = np.asarray(x)
    in_maps = []
    for c in range(NCORES):
        m = dict(consts)
        m["idx"] = _pack_idx(x[c * BL:(c + 1) * BL, :])
        in_maps.append(m)
    return in_maps


def kernel(x, lengths, emb, W_ih_f, W_hh_f, W_ih_b, W_hh_b, W_fc, b_fc):
    in_maps = _make_in_maps(x, emb, W_ih_f, W_hh_f, W_ih_b, W_hh_b, W_fc, b_fc)
    results = _get_runner()(in_maps)
    out = np.concatenate(
        [np.ascontiguousarray(results[c]["outT"].T) for c in range(NCORES)],
        axis=0,
    ).astype(np.float32)
    return out
